# revision 14
# baseline (speedup 1.0000x reference)
"""Multi-head attention (B=4, S=2048, E=1024, H=16, D=64) on 8 TRN2 NeuronCores.

The metric is dominated by host->device transfer bytes, so inputs are
uploaded in bf16 with zero duplication (54 MB total vs 252 MB fp32
baseline): core c (b=c//2, g=c%2) uploads

  - x_q/x_k/x_v[b, seq-half g]            [1024, 1024] bf16 (natural [s, e])
  - W*[:, 512g + 128b : 512g + 128(b+1)]  [1024, 128]  bf16 (its 1/8 shard)

On-chip collectives reassemble what each core needs:
  - AllGather over quads {g, g+2, g+4, g+6}: full W head-group half per core
  - AllGather over pairs {2b, 2b+1}: both seq halves of batch b per core

Per-core compute (bf16 matmuls, fp32 accumulation):
  xT via transposing DMA; qT/kT = W^T @ xT; v = xT^T @ W (natural), with an
  appended ones column so the ctx matmul also produces softmax denominators;
  per head: S^T = kT^T q chunks, P = exp(S^T/8) (bf16), ctx^T accumulated in
  PSUM; denominators reciprocal'd on ACT, broadcast across partitions with a
  rank-1 matmul, multiplied in on DVE; output written as ctx^T [512, 2048]
  bf16 (2 MB/core) and transposed+cast on host.
"""

import numpy as np
import ml_dtypes
from contextlib import ExitStack

import concourse.bass as bass
import concourse.tile as tile
from concourse import bacc
from concourse import mybir
from concourse.bass_utils import run_bass_kernel_spmd

F32 = mybir.dt.float32
F32R = mybir.dt.float32r
BF16 = mybir.dt.bfloat16
EXP = mybir.ActivationFunctionType.Exp
RECIP = mybir.ActivationFunctionType.Reciprocal
BF = ml_dtypes.bfloat16

B, S, E = 4, 2048, 1024
H, D = 16, 64
HPC = 8             # heads per core
FPC = HPC * D       # 512 features per core
SH = S // 2         # seq half
N_CORES = 8
KC = E // 128       # contraction chunks
NT = S // 128       # k blocks
NJ = S // 512       # q chunks
SCALE = 0.125       # 1/sqrt(64)

PAIRS = [[0, 1], [2, 3], [4, 5], [6, 7]]
QUADS = [[0, 2, 4, 6], [1, 3, 5, 7]]


def build_bass():
    nc = bacc.Bacc(num_devices=N_CORES)
    xq = nc.declare_dram_parameter("xq", [SH, E], BF16, isOutput=False)
    xk = nc.declare_dram_parameter("xk", [SH, E], BF16, isOutput=False)
    xv = nc.declare_dram_parameter("xv", [SH, E], BF16, isOutput=False)
    wq = nc.declare_dram_parameter("wq", [E, 128], BF16, isOutput=False)
    wk = nc.declare_dram_parameter("wk", [E, 128], BF16, isOutput=False)
    wv = nc.declare_dram_parameter("wv", [E, 128], BF16, isOutput=False)
    out = nc.declare_dram_parameter("out", [FPC, S], BF16, isOutput=True)

    with tile.TileContext(nc) as tc, ExitStack() as ctx:
        dram = ctx.enter_context(tc.tile_pool(name="dram", bufs=1, space="DRAM"))
        sb = ctx.enter_context(tc.tile_pool(name="sb", bufs=1))
        exp_pool = ctx.enter_context(tc.tile_pool(name="expp", bufs=2))
        fin = ctx.enter_context(tc.tile_pool(name="fin", bufs=2))

        # ---- bounce buffers + collectives ----
        win_b = dram.tile([3, E, 128], BF16, name="win_b")
        wout_b = dram.tile([4, 3, E, 128], BF16, name="wout_b")
        xin_b = dram.tile([3, SH, E], BF16, name="xin_b")
        xout_b = dram.tile([2, 3, SH, E], BF16, name="xout_b")

        # Collective completion is signaled via semaphore, NOT via normal
        # instruction dependencies — every reader of a gathered buffer must
        # explicitly wait_ge on the collective's semaphore.
        sem_w = nc.alloc_semaphore(name="cc_w")
        sem_x = nc.alloc_semaphore(name="cc_x")
        tok_w = sb.tile([1, 4], BF16, name="tok_w", tag="tok")
        tok_x = sb.tile([1, 4], BF16, name="tok_x", tag="tok")
        for i, t in enumerate((wq, wk, wv)):
            nc.gpsimd.dma_start(out=win_b[i], in_=t[:, :])
        nc.gpsimd.collective_compute(
            "AllGather", mybir.AluOpType.bypass, replica_groups=QUADS,
            ins=[win_b.opt()], outs=[wout_b.opt()])
        # The triggering engine's next instruction runs only after the
        # collective's internal completion wait, so a probe DMA reading the
        # gathered buffer publishes "data landed" to other engines.
        nc.gpsimd.dma_start(out=tok_w, in_=wout_b[3, 2][E - 1:E, 124:128]
                            ).then_inc(sem_w, 16)
        for i, t in enumerate((xq, xk, xv)):
            nc.gpsimd.dma_start(out=xin_b[i], in_=t[:, :])
        nc.gpsimd.collective_compute(
            "AllGather", mybir.AluOpType.bypass, replica_groups=PAIRS,
            ins=[xin_b.opt()], outs=[xout_b.opt()])
        nc.gpsimd.dma_start(out=tok_x, in_=xout_b[1, 2][SH - 1:SH, E - 4:E]
                            ).then_inc(sem_x, 16)

        # ---- W into SBUF: w_sb[t] [128, KC, FPC], f = 128*quad_rank + col ----
        w_sb = []
        for t in range(3):
            wt = sb.tile([128, KC, FPC], BF16, name=f"w_sb{t}", tag=f"w_sb{t}")
            for shard in range(4):
                nc.sync.dma_start(
                    out=wt[:, :, shard * 128:(shard + 1) * 128],
                    in_=wout_b[shard, t].rearrange("(kc p) c -> p kc c", p=128),
                )._wait_ge(sem_w, 16)
            w_sb.append(wt)

        # ---- xT[t] [128, KC, S] via transposing DMA ----
        xT = []
        eng = [nc.sync, nc.scalar]
        n = 0
        for t in range(3):
            xt = sb.tile([128, KC, S], BF16, name=f"xT{t}", tag=f"xT{t}")
            for half in range(2):
                for ec in range(KC):
                    nc.sync.dma_start_transpose(
                        out=xt[:, ec, half * SH:(half + 1) * SH],
                        in_=xout_b[half, t][:, ec * 128:(ec + 1) * 128],
                    )._wait_ge(sem_x, 16)
                    n += 1
            xT.append(xt)

        # ---- projections ----
        qT = sb.tile([128, 4, S], BF16, name="qT", tag="qT")
        kT = sb.tile([128, 4, S], BF16, name="kT", tag="kT")
        v_aug = sb.tile([128, HPC, NT, D + 1], BF16, name="v_aug", tag="v_aug")
        nc.vector.memset(v_aug, 1.0)  # col D stays 1.0; cols 0..D-1 overwritten

        with tc.tile_pool(name="psp", bufs=3, space="PSUM") as psp:
            for xi, dst in ((0, qT), (1, kT)):
                for fc in range(4):
                    for scj in range(4):
                        acc = psp.tile([128, 512], F32, name=f"p{xi}_{fc}_{scj}",
                                       tag="proj")
                        for ecc in range(KC):
                            nc.tensor.matmul(
                                acc,
                                lhsT=w_sb[xi][:, ecc, fc * 128:(fc + 1) * 128],
                                rhs=xT[xi][:, ecc, scj * 512:(scj + 1) * 512],
                                start=(ecc == 0), stop=(ecc == KC - 1))
                        nc.vector.tensor_copy(
                            out=dst[:, fc, scj * 512:(scj + 1) * 512], in_=acc)
            for sc in range(NT):
                acc = psp.tile([128, FPC], F32, name=f"pv_{sc}", tag="proj")
                for ecc in range(KC):
                    nc.tensor.matmul(
                        acc,
                        lhsT=xT[2][:, ecc, sc * 128:(sc + 1) * 128],
                        rhs=w_sb[2][:, ecc, :],
                        start=(ecc == 0), stop=(ecc == KC - 1))
                for h in range(HPC):
                    nc.vector.tensor_copy(
                        out=v_aug[:, h, sc, 0:D], in_=acc[:, h * D:(h + 1) * D])

        # ---- attention ----
        ones_col = sb.tile([1, D], F32, name="ones_col", tag="ones")
        nc.vector.memset(ones_col, 1.0)
        with tc.tile_pool(name="stp", bufs=2, space="PSUM") as stp, \
             tc.tile_pool(name="cac", bufs=2, space="PSUM") as cac, \
             tc.tile_pool(name="rbp", bufs=2, space="PSUM") as rbp:
            for h in range(HPC):
                po = (h % 2) * 64
                fc = h // 2
                for j in range(NJ):
                    cacc = cac.tile([D + 1, 512], F32, name=f"c{h}_{j}", tag="ctx")
                    for tg in range(NT // 2):
                        st = stp.tile([128, 2, 512], F32, name=f"st{h}_{j}_{tg}",
                                      tag="st")
                        for u in range(2):
                            t = tg * 2 + u
                            nc.tensor.matmul(
                                st[:, u],
                                lhsT=kT[po:po + 64, fc, t * 128:(t + 1) * 128],
                                rhs=qT[po:po + 64, fc, j * 512:(j + 1) * 512],
                                start=True, stop=True)
                        ex = exp_pool.tile([128, 2, 512], BF16,
                                           name=f"ex{h}_{j}_{tg}", tag="ex")
                        nc.scalar.activation(ex, st, EXP, scale=SCALE)
                        for u in range(2):
                            t = tg * 2 + u
                            nc.tensor.matmul(
                                cacc, lhsT=v_aug[:, h, t, :], rhs=ex[:, u],
                                start=(t == 0), stop=(t == NT - 1))
                    # finalize: ctx / den, bf16, ctx^T layout out
                    rec = fin.tile([1, 512], F32, name=f"r{h}_{j}", tag="rec")
                    nc.vector.reciprocal(rec, cacc[D:D + 1, :])
                    rb = rbp.tile([D, 512], F32, name=f"rb{h}_{j}", tag="rb")
                    nc.tensor.matmul(rb, lhsT=ones_col[:, :], rhs=rec[:, :],
                                     start=True, stop=True)
                    csb = fin.tile([D, 512], F32, name=f"cs{h}_{j}", tag="csb")
                    nc.vector.tensor_copy(out=csb, in_=cacc[0:D, :])
                    cbf = fin.tile([D, 512], BF16, name=f"cb{h}_{j}", tag="cbf")
                    nc.vector.tensor_tensor(cbf, csb, rb, mybir.AluOpType.mult)
                    nc.sync.dma_start(
                        out=out[h * D:(h + 1) * D, j * 512:(j + 1) * 512],
                        in_=cbf)

    nc.compile()
    nc.freeze()
    return nc


_NC_CACHE = None


def _get_nc():
    global _NC_CACHE
    if _NC_CACHE is None:
        _NC_CACHE = build_bass()
    return _NC_CACHE


def _prep_in_maps(inputs):
    q32 = np.asarray(inputs["queries"], np.float32)
    k32 = np.asarray(inputs["keys"], np.float32)
    v32 = np.asarray(inputs["values"], np.float32)
    Wq = np.asarray(inputs["Wq"], np.float32)
    Wk = np.asarray(inputs["Wk"], np.float32)
    Wv = np.asarray(inputs["Wv"], np.float32)

    in_maps = []
    for c in range(N_CORES):
        b, g = c // 2, c % 2
        lo = g * FPC + b * 128
        in_maps.append({
            "xq": q32[b, g * SH:(g + 1) * SH].astype(BF),
            "xk": k32[b, g * SH:(g + 1) * SH].astype(BF),
            "xv": v32[b, g * SH:(g + 1) * SH].astype(BF),
            "wq": np.ascontiguousarray(Wq[:, lo:lo + 128]).astype(BF),
            "wk": np.ascontiguousarray(Wk[:, lo:lo + 128]).astype(BF),
            "wv": np.ascontiguousarray(Wv[:, lo:lo + 128]).astype(BF),
        })
    return in_maps


def kernel(queries, keys, values, Wq, Wk, Wv, **_):
    in_maps = _prep_in_maps(dict(queries=queries, keys=keys, values=values,
                                 Wq=Wq, Wk=Wk, Wv=Wv))
    nc = _get_nc()
    res = run_bass_kernel_spmd(nc, in_maps, list(range(N_CORES)))

    full = np.empty((B, S, H * D), dtype=np.float32)
    for c in range(N_CORES):
        b, g = c // 2, c % 2
        o = res.results[c]["out"]  # [FPC, S] bf16, ctx^T
        full[b, :, g * FPC:(g + 1) * FPC] = o.astype(np.float32).T
    return full


# revision 15
# speedup vs baseline: 1.2773x; 1.2773x over previous
"""Multi-head attention (B=4, S=2048, E=1024, H=16, D=64) on 8 TRN2 NeuronCores.

The metric is dominated by host->device transfer bytes, so inputs are
uploaded in bf16 with zero duplication (54 MB total vs 252 MB fp32
baseline): core c (b=c//2, g=c%2) uploads

  - x_q/x_k/x_v[b, seq-half g]            [1024, 1024] bf16 (natural [s, e])
  - W*[:, 512g + 128b : 512g + 128(b+1)]  [1024, 128]  bf16 (its 1/8 shard)

On-chip collectives reassemble what each core needs:
  - AllGather over quads {g, g+2, g+4, g+6}: full W head-group half per core
  - AllGather over pairs {2b, 2b+1}, one per tensor (k, q, v) so transposes
    and projections of earlier tensors overlap the later gathers.

Collective completion is only observable through an explicit semaphore:
a probe DMA on the triggering engine (whose queue drains after the
collective's internal completion wait) then_incs it; every cross-engine
reader of gathered data carries a _wait_ge. Transposing DMAs all stay on
one queue -- concurrent XBAR transposes from two queues corrupt data.

Per-core compute (bf16 matmuls, fp32 accumulation):
  xT via transposing DMA; qT/kT = W^T @ xT; v = xT^T @ W (natural), with an
  appended ones column so the ctx matmul also produces softmax denominators;
  per head: S^T = kT^T q chunks, P = exp(S^T/8) (bf16), ctx^T accumulated in
  PSUM; denominators reciprocal'd on DVE, broadcast across partitions on
  gpsimd, multiplied against PSUM on DVE; output written as ctx^T
  [512, 2048] bf16 (2 MB/core) and transposed+cast on host.
"""

import numpy as np
import ml_dtypes
from contextlib import ExitStack

import concourse.bass as bass
import concourse.tile as tile
from concourse import bacc
from concourse import mybir
from concourse.bass_utils import run_bass_kernel_spmd

F32 = mybir.dt.float32
BF16 = mybir.dt.bfloat16
EXP = mybir.ActivationFunctionType.Exp
BF = ml_dtypes.bfloat16

B, S, E = 4, 2048, 1024
H, D = 16, 64
HPC = 8             # heads per core
FPC = HPC * D       # 512 features per core
SH = S // 2         # seq half
N_CORES = 8
KC = E // 128       # contraction chunks
NT = S // 128       # k blocks
NJ = S // 512       # q chunks
SCALE = 0.125       # 1/sqrt(64)

PAIRS = [[0, 1], [2, 3], [4, 5], [6, 7]]
QUADS = [[0, 2, 4, 6], [1, 3, 5, 7]]


def build_bass():
    nc = bacc.Bacc(num_devices=N_CORES)
    xq = nc.declare_dram_parameter("xq", [SH, E], BF16, isOutput=False)
    xk = nc.declare_dram_parameter("xk", [SH, E], BF16, isOutput=False)
    xv = nc.declare_dram_parameter("xv", [SH, E], BF16, isOutput=False)
    wq = nc.declare_dram_parameter("wq", [E, 128], BF16, isOutput=False)
    wk = nc.declare_dram_parameter("wk", [E, 128], BF16, isOutput=False)
    wv = nc.declare_dram_parameter("wv", [E, 128], BF16, isOutput=False)
    out = nc.declare_dram_parameter("out", [FPC, S], BF16, isOutput=True)

    with tile.TileContext(nc) as tc, ExitStack() as ctx:
        dram = ctx.enter_context(tc.tile_pool(name="dram", bufs=1, space="DRAM"))
        sb = ctx.enter_context(tc.tile_pool(name="sb", bufs=1))
        exp_pool = ctx.enter_context(tc.tile_pool(name="expp", bufs=3))
        fin = ctx.enter_context(tc.tile_pool(name="fin", bufs=2))

        # ---- bounce buffers + collectives ----
        win_b = dram.tile([3, E, 128], BF16, name="win_b")
        wout_b = dram.tile([4, 3, E, 128], BF16, name="wout_b")
        # x slot order: k, q, v — k gathers first (kT must be fully projected
        # before any score matmul), v last (ctx needs it latest).
        xin_b = dram.tile([3, SH, E], BF16, name="xin_b")
        xout_b = dram.tile([3, 2, SH, E], BF16, name="xout_b")

        sem_w = nc.alloc_semaphore(name="cc_w")
        sems_x = [nc.alloc_semaphore(name=f"cc_x{i}") for i in range(3)]
        tok = sb.tile([1, 4, 4], BF16, name="tok", tag="tok")

        for i, t in enumerate((wq, wk, wv)):
            nc.scalar.dma_start(out=win_b[i], in_=t[:, :])
        nc.gpsimd.collective_compute(
            "AllGather", mybir.AluOpType.bypass, replica_groups=QUADS,
            ins=[win_b.opt()], outs=[wout_b.opt()])
        nc.gpsimd.dma_start(out=tok[:, 3], in_=wout_b[3, 2][E - 1:E, 124:128]
                            ).then_inc(sem_w, 16)

        xorder = (xk, xq, xv)  # gather order; index in this tuple = slot
        for i, t in enumerate(xorder):
            nc.scalar.dma_start(out=xin_b[i], in_=t[:, :])
        for i in range(3):
            nc.gpsimd.collective_compute(
                "AllGather", mybir.AluOpType.bypass, replica_groups=PAIRS,
                ins=[xin_b[i].opt()], outs=[xout_b[i].opt()])
            nc.gpsimd.dma_start(out=tok[:, i], in_=xout_b[i, 1][SH - 1:SH, E - 4:E]
                                ).then_inc(sems_x[i], 16)

        # ---- W into SBUF: w_sb[t] [128, KC, FPC], f = 128*quad_rank + col ----
        w_sb = []
        for t in range(3):
            wt = sb.tile([128, KC, FPC], BF16, name=f"w_sb{t}", tag=f"w_sb{t}")
            for shard in range(4):
                nc.sync.dma_start(
                    out=wt[:, :, shard * 128:(shard + 1) * 128],
                    in_=wout_b[shard, t].rearrange("(kc p) c -> p kc c", p=128),
                )._wait_ge(sem_w, 16)
            w_sb.append(wt)

        # ---- xT via transposing DMA (slot order k, q, v) ----
        # ALL transposes on one queue: concurrent XBAR transposes corrupt.
        xT = []
        for i in range(3):
            xt = sb.tile([128, KC, S], BF16, name=f"xT{i}", tag=f"xT{i}")
            for half in range(2):
                for ec in range(KC):
                    nc.sync.dma_start_transpose(
                        out=xt[:, ec, half * SH:(half + 1) * SH],
                        in_=xout_b[i, half][:, ec * 128:(ec + 1) * 128],
                    )._wait_ge(sems_x[i], 16)
            xT.append(xt)
        xT_k, xT_q, xT_v = xT

        # ---- projections ----
        qT = sb.tile([128, 4, S], BF16, name="qT", tag="qT")
        kT = sb.tile([128, 4, S], BF16, name="kT", tag="kT")
        v_aug = sb.tile([128, HPC, NT, D + 1], BF16, name="v_aug", tag="v_aug")
        nc.vector.memset(v_aug, 1.0)  # col D stays 1.0; cols 0..D-1 overwritten

        with tc.tile_pool(name="psp", bufs=3, space="PSUM") as psp:
            for xi, xs, wi, dst in ((0, xT_k, 1, kT), (1, xT_q, 0, qT)):
                for fc in range(4):
                    for scj in range(4):
                        acc = psp.tile([128, 512], F32, name=f"p{xi}_{fc}_{scj}",
                                       tag="proj")
                        for ecc in range(KC):
                            nc.tensor.matmul(
                                acc,
                                lhsT=w_sb[wi][:, ecc, fc * 128:(fc + 1) * 128],
                                rhs=xs[:, ecc, scj * 512:(scj + 1) * 512],
                                start=(ecc == 0), stop=(ecc == KC - 1))
                        nc.vector.tensor_copy(
                            out=dst[:, fc, scj * 512:(scj + 1) * 512], in_=acc)
            for sc in range(NT):
                acc = psp.tile([128, FPC], F32, name=f"pv_{sc}", tag="proj")
                for ecc in range(KC):
                    nc.tensor.matmul(
                        acc,
                        lhsT=xT_v[:, ecc, sc * 128:(sc + 1) * 128],
                        rhs=w_sb[2][:, ecc, :],
                        start=(ecc == 0), stop=(ecc == KC - 1))
                for h in range(HPC):
                    nc.vector.tensor_copy(
                        out=v_aug[:, h, sc, 0:D], in_=acc[:, h * D:(h + 1) * D])

        # ---- attention ----
        with tc.tile_pool(name="stp", bufs=3, space="PSUM") as stp, \
             tc.tile_pool(name="cac", bufs=2, space="PSUM") as cac:
            for h in range(HPC):
                po = (h % 2) * 64
                fc = h // 2
                for j in range(NJ):
                    cacc = cac.tile([D + 1, 512], F32, name=f"c{h}_{j}", tag="ctx")
                    for tg in range(NT // 2):
                        st = stp.tile([128, 2, 512], F32, name=f"st{h}_{j}_{tg}",
                                      tag="st")
                        for u in range(2):
                            t = tg * 2 + u
                            nc.tensor.matmul(
                                st[:, u],
                                lhsT=kT[po:po + 64, fc, t * 128:(t + 1) * 128],
                                rhs=qT[po:po + 64, fc, j * 512:(j + 1) * 512],
                                start=True, stop=True)
                        ex = exp_pool.tile([128, 2, 512], BF16,
                                           name=f"ex{h}_{j}_{tg}", tag="ex")
                        nc.scalar.activation(ex, st, EXP, scale=SCALE)
                        for u in range(2):
                            t = tg * 2 + u
                            nc.tensor.matmul(
                                cacc, lhsT=v_aug[:, h, t, :], rhs=ex[:, u],
                                start=(t == 0), stop=(t == NT - 1))
                    # finalize: ctx / den -> bf16, ctx^T layout out
                    rec = fin.tile([1, 512], F32, name=f"r{h}_{j}", tag="rec")
                    nc.vector.reciprocal(rec, cacc[D:D + 1, :])
                    rb = fin.tile([D, 512], F32, name=f"rb{h}_{j}", tag="rb")
                    nc.gpsimd.partition_broadcast(rb, rec)
                    cbf = fin.tile([D, 512], BF16, name=f"cb{h}_{j}", tag="cbf")
                    nc.vector.tensor_tensor(cbf, cacc[0:D, :], rb,
                                            mybir.AluOpType.mult)
                    nc.sync.dma_start(
                        out=out[h * D:(h + 1) * D, j * 512:(j + 1) * 512],
                        in_=cbf)

    nc.compile()
    nc.freeze()
    return nc


_NC_CACHE = None


def _get_nc():
    global _NC_CACHE
    if _NC_CACHE is None:
        _NC_CACHE = build_bass()
    return _NC_CACHE


def _prep_in_maps(inputs):
    q32 = np.asarray(inputs["queries"], np.float32)
    k32 = np.asarray(inputs["keys"], np.float32)
    v32 = np.asarray(inputs["values"], np.float32)
    Wq = np.asarray(inputs["Wq"], np.float32)
    Wk = np.asarray(inputs["Wk"], np.float32)
    Wv = np.asarray(inputs["Wv"], np.float32)

    in_maps = []
    for c in range(N_CORES):
        b, g = c // 2, c % 2
        lo = g * FPC + b * 128
        in_maps.append({
            "xq": q32[b, g * SH:(g + 1) * SH].astype(BF),
            "xk": k32[b, g * SH:(g + 1) * SH].astype(BF),
            "xv": v32[b, g * SH:(g + 1) * SH].astype(BF),
            "wq": np.ascontiguousarray(Wq[:, lo:lo + 128]).astype(BF),
            "wk": np.ascontiguousarray(Wk[:, lo:lo + 128]).astype(BF),
            "wv": np.ascontiguousarray(Wv[:, lo:lo + 128]).astype(BF),
        })
    return in_maps


def kernel(queries, keys, values, Wq, Wk, Wv, **_):
    in_maps = _prep_in_maps(dict(queries=queries, keys=keys, values=values,
                                 Wq=Wq, Wk=Wk, Wv=Wv))
    nc = _get_nc()
    res = run_bass_kernel_spmd(nc, in_maps, list(range(N_CORES)))

    full = np.empty((B, S, H * D), dtype=np.float32)
    for c in range(N_CORES):
        b, g = c // 2, c % 2
        o = res.results[c]["out"]  # [FPC, S] bf16, ctx^T
        full[b, :, g * FPC:(g + 1) * FPC] = o.astype(np.float32).T
    return full
